# revision 42
# baseline (speedup 1.0000x reference)
"""Trainium2 Bass kernel for nn_MultiHeadAttention_72877005078903.

MHA with ALiBi + causal mask: B=2, T=2048, D=1024, 16 heads, head_dim 64.

Sharding (8 cores): batch x head-quartile. Core c in {0..3} handles batch 0,
cores 4..7 batch 1. Within a batch group, core c owns heads {c, c+4, c+8, c+12}
(one head per ALiBi-slope quartile) so every core's 4 head-slots share the same
per-slot causal/ALiBi block window, keeping the SPMD program identical across
cores while balancing work. Slot s holds head c + 4*(3-s): slot 0 has the
smallest slope (biggest kept window) so its scores/exp start right after the
first projection pass.

Device-side structure:
- Q/K projections run in fp8(e4m3) DoubleRow mode (0.5 PE cycles/column);
  weights carry a x64 pre-scale to keep fp8 mantissas normal, undone in the
  fused PSUM evacuation. V and the output projection stay bf16 (their
  quantization error would pass straight to the output).
- Scores are computed transposed (S.T[j,i]) in fp32r (1 PE cycle/column for
  pieces >= 256 wide) so the exp output in SBUF is directly the lhsT of the
  attention*V matmul (no transposes in the hot loop).
- ALiBi bias slope*(j-i) is fused into the score matmul as four extra
  contraction rows: slope*pos split into a bf16-exact hi part + small lo
  residual per side, so the fp32r (~17-bit) matmul keeps the cancellation
  exact to ~1e-4: lhsT=[kT; Ahi; Alo; 1; 1], rhs=[qT; 1; 1; Bhi; Blo].
- Emission order = per-engine queue order: projections first, score/exp
  phases of the big-window slots overlap the second projection pass on the
  scalar engine, context phases run interleaved across slots per row-block
  with the output projection fused behind them.
- A ones-column appended to V makes the softmax denominator fall out of the
  same PSUM accumulation (no max-subtraction needed: scores are O(1) and the
  ALiBi bias is <= 0 on the causal side).
- Per-slot block window: blocks with slope*(distance) >= ~12 contribute < 1e-4
  of the row mass (for the worst head in the slot) and are skipped.
- Causal mask on the diagonal block is applied additively (-1e30) on the PSUM
  scores before exp, avoiding inf*0 NaNs.
- The context is transposed per 128-block on the PE (identity matmul) and fed
  straight into the output projection; partials are summed on the host, where
  bp + bv @ Wp.T is also folded in.
"""

import numpy as np
import ml_dtypes

B, T, D, NH, HD = 2, 2048, 1024, 16, 64
HPC = 4                      # head-slots per core
NB = T // 128                # 16 row blocks
NKK = D // 128               # 8 contraction tiles for projections
NTC = 4                      # 512-wide t-chunks
D_KEEP = (16, 7, 3, 2)       # per-slot kept block-diagonal width
SCALE = float(D) ** 0.25
BF16 = ml_dtypes.bfloat16
NEG = -1.0e30

_PROG = None


def _build_program(nrep=1, fuse_out=True):
    """nrep>1 repeats the whole kernel body back-to-back in one NEFF —
    used only to measure device time as a wall-clock delta."""
    import concourse.bacc as bacc
    import concourse.tile as tile
    from concourse import mybir

    f32 = mybir.dt.float32
    f32r = mybir.dt.float32r
    bf16 = mybir.dt.bfloat16
    Exp = mybir.ActivationFunctionType.Exp
    Copy = mybir.ActivationFunctionType.Copy
    Ident = mybir.ActivationFunctionType.Identity

    fp8 = mybir.dt.float8e4
    DR = mybir.MatmulPerfMode.DoubleRow

    nc = bacc.Bacc()
    x8_d = nc.declare_dram_parameter("x8", [D, T], fp8, isOutput=False)
    xd8_d = nc.declare_dram_parameter("xd8", [D, T], fp8, isOutput=False)
    wq_d = nc.declare_dram_parameter("wq", [D, 256], fp8, isOutput=False)
    wk_d = nc.declare_dram_parameter("wk", [D, 256], fp8, isOutput=False)
    wv_d = nc.declare_dram_parameter("wv", [D, 256], fp8, isOutput=False)
    wvd_d = nc.declare_dram_parameter("wvd", [D, 256], fp8, isOutput=False)
    wp_d = nc.declare_dram_parameter("wp", [256, D], bf16, isOutput=False)
    bq_d = nc.declare_dram_parameter("bq", [128, HPC], f32, isOutput=False)
    bk_d = nc.declare_dram_parameter("bk", [128, HPC], f32, isOutput=False)
    qr_d = nc.declare_dram_parameter("qrows", [HPC, 4, T], f32r, isOutput=False)
    kr_d = nc.declare_dram_parameter("krows", [HPC, 4, T], f32r, isOutput=False)
    mk_d = nc.declare_dram_parameter("maskadd", [128, 128], bf16, isOutput=False)
    id_d = nc.declare_dram_parameter("ident", [128, 128], bf16, isOutput=False)
    out_d = nc.declare_dram_parameter("out", [T, D], bf16, isOutput=True)

    from contextlib import ExitStack

    with tile.TileContext(nc) as tc:
        with ExitStack() as ctx:
            consts = ctx.enter_context(tc.tile_pool(name="consts", bufs=1))
            qkp = ctx.enter_context(tc.tile_pool(name="qkp", bufs=1))
            vp = ctx.enter_context(tc.tile_pool(name="vp", bufs=1))
            ptp = ctx.enter_context(tc.tile_pool(name="ptp", bufs=33))
            cxp = ctx.enter_context(tc.tile_pool(name="cxp", bufs=1))
            small = ctx.enter_context(tc.tile_pool(name="small", bufs=8))
            obp = ctx.enter_context(tc.tile_pool(name="obp", bufs=2))
            ctx_pools = {"xd8p": ctx.enter_context(
                tc.tile_pool(name="xd8p", bufs=2)),
                "x8db": ctx.enter_context(
                tc.tile_pool(name="x8db", bufs=2))}
            ps_m = ctx.enter_context(tc.tile_pool(name="ps_m", bufs=2, space="PSUM"))

            def emit_once():
                # ---- inputs to SBUF ----
                # wq first (first matmul needs it), then xT t-chunk-major so the
                # first projection pass streams at DMA arrival rate.
                # fp8 Q/K-projection operands first: the DoubleRow
                # matmuls are the head of the PE stream
                wq_s = consts.tile([128, NKK, 256], fp8, name="wq_s", tag="wq_s")
                nc.sync.dma_start(wq_s[:], wq_d.rearrange("(o p) m -> p o m", p=128))
                bq_s = consts.tile([128, HPC], f32, name="bq_s", tag="bq_s")
                nc.sync.dma_start(bq_s[:], bq_d[:])
                bk_s = consts.tile([128, HPC], f32, name="bk_s", tag="bk_s")
                nc.sync.dma_start(bk_s[:], bk_d[:])
                x8 = []
                for tcq in range(NTC):
                    # tcq0/1 double-buffered: the next repetition's stream-in
                    # overlaps this one's attention tail (cross-body
                    # pipelining in steady state)
                    pool = ctx_pools["x8db"] if tcq < 2 else consts
                    t = pool.tile([128, NKK, 512], fp8, name=f"x8_{tcq}",
                                  tag=f"x8_{tcq}")
                    nc.sync.dma_start(
                        t[:], x8_d[:, 512 * tcq:512 * (tcq + 1)]
                        .rearrange("(o p) m -> p o m", p=128))
                    x8.append(t)
                # fp8 residual of x (V-projection correction), needed
                # only by proj_v: DMA'd late so the startup stream is just
                # x8 + small weights
                xd8p = ctx_pools["xd8p"]
                xd8 = [xd8p.tile([128, NKK, 1024], fp8, name=f"xd8_{tc2}",
                                 tag="xd8")
                       for tc2 in range(2)]
                # other weights on the scalar-engine HWDGE queue (parallel
                # to sync; gpsimd SWDGE has ~1us per-DMA overhead)
                wk_s = consts.tile([128, NKK, 256], fp8, name="wk_s", tag="wk_s")
                nc.scalar.dma_start(wk_s[:], wk_d.rearrange("(o p) m -> p o m", p=128))
                wv_s = consts.tile([128, NKK, 256], fp8, name="wv_s", tag="wv_s")
                wvd_s = consts.tile([128, NKK, 256], fp8, name="wvd_s",
                                    tag="wvd_s")
                zbias = consts.tile([128, 1], f32, name="zbias", tag="zbias")
                nc.vector.memset(zbias[:], 0.0)
                mcl_s = consts.tile([128, 128], bf16, name="mcl_s", tag="mcl_s")
                id_s = consts.tile([128, 128], bf16, name="id_s", tag="id_s")
                wp_s = consts.tile([128, 2, D], bf16, name="wp_s", tag="wp_s")

                def late_dmas():
                    # emitted after the m0 projections so these don't occupy
                    # the serial HWDGE resource during the x stream-in
                    nc.scalar.dma_start(
                        wv_s[:], wv_d.rearrange("(o p) m -> p o m", p=128))
                    nc.scalar.dma_start(
                        wvd_s[:], wvd_d.rearrange("(o p) m -> p o m", p=128))
                    for tc2 in range(2):
                        nc.sync.dma_start(
                            xd8[tc2][:],
                            xd8_d[:, 1024 * tc2:1024 * (tc2 + 1)]
                            .rearrange("(o p) m -> p o m", p=128))
                    nc.scalar.dma_start(mcl_s[:], mk_d[:])
                    nc.scalar.dma_start(id_s[:], id_d[:])
                    nc.scalar.dma_start(
                        wp_s[:], wp_d.rearrange("(o p) m -> p o m", p=128))

                # Head-slot q/k layout (fp32r, [128, T] tiles):
                #  even slot: rows 0:64 = qT/kT data, rows 64:68 = alibi rows;
                #             score matmuls contract over partitions [0:68].
                #  odd slot:  rows 0:60 zeroed, rows 60:64 = alibi rows,
                #             rows 64:128 = qT/kT data (same lanes as the PSUM
                #             half it's copied from); contract over [0:128]
                #             (SBUF APs >32 partitions must start at 0 or 64).
                # 4 alibi rows per side: slope*pos split into a bf16-exact hi
                # part + small lo residual so the fp32r (~17-bit) matmul keeps
                # the slope*(j-i) cancellation exact to ~1e-4.
                q_att, k_att = [], []
                for s in range(HPC):
                    qa = qkp.tile([128, T], f32r, name=f"qa{s}", tag=f"qa{s}")
                    ka = qkp.tile([128, T], f32r, name=f"ka{s}", tag=f"ka{s}")
                    if s % 2 != 0:
                        # Pool memset can't emit f32r; zeros are exact in
                        # either view so memset through an f32 bitcast
                        nc.gpsimd.memset(qa[0:60, :].bitcast(f32), 0.0)
                        nc.gpsimd.memset(ka[0:60, :].bitcast(f32), 0.0)
                    q_att.append(qa)
                    k_att.append(ka)

                def alibi_dmas(s):
                    r = slice(64, 68) if s % 2 == 0 else slice(60, 64)
                    nc.scalar.dma_start(q_att[s][r, :], qr_d[s])
                    nc.scalar.dma_start(k_att[s][r, :], kr_d[s])
                v4 = vp.tile([128, NB, HPC, 65], bf16, name="v4", tag="v4")
                nc.gpsimd.memset(v4[:, :, :, 64:65], 1.0)
                ctx_sb = cxp.tile([128, NB, 256], bf16, name="ctx_sb", tag="ctx_sb")

                # ---- projections ----
                def proj_qk_tc(w_s, b_s, att, m, tcq):
                    # fp8 DoubleRow: one matmul contracts 2 128-row k-tiles
                    # at 0.5 PE cycles per output column. Weights carry a
                    # x64 pre-scale (keeps fp8 mantissas in the normal
                    # range); undone in the fused evacuation below.
                    tsl = slice(512 * tcq, 512 * (tcq + 1))
                    ps = ps_m.tile([128, 512], f32, name="psmm", tag="psmm")
                    for kk2 in range(0, NKK, 2):
                        nc.tensor.matmul(
                            ps[:],
                            w_s[:, kk2:kk2 + 2, 128 * m:128 * (m + 1)],
                            x8[tcq][:, kk2:kk2 + 2, :],
                            start=(kk2 == 0),
                            stop=(kk2 == NKK - 2),
                            perf_mode=DR,
                        )
                    s_ev, s_od = 2 * m, 2 * m + 1
                    if m == 0:
                        # Act is idle during the first projection pass and
                        # DVE is the gate there: evacuate m0 (and half of
                        # m1) on Act via Identity(in*1/64 + bias)
                        nc.scalar.activation(
                            att[s_ev][0:64, tsl], ps[0:64, :], Ident,
                            bias=b_s[0:64, s_ev:s_ev + 1], scale=1.0 / 64.0)
                        nc.scalar.activation(
                            att[s_od][64:128, tsl], ps[64:128, :], Ident,
                            bias=b_s[64:128, s_od:s_od + 1], scale=1.0 / 64.0)
                    else:
                        nc.vector.tensor_scalar(
                            att[s_ev][0:64, tsl], ps[0:64, :],
                            1.0 / 64.0, b_s[0:64, s_ev:s_ev + 1],
                            mybir.AluOpType.mult, mybir.AluOpType.add)
                        nc.vector.tensor_scalar(
                            att[s_od][64:128, tsl], ps[64:128, :],
                            1.0 / 64.0, b_s[64:128, s_od:s_od + 1],
                            mybir.AluOpType.mult, mybir.AluOpType.add)

                def proj_v():
                    # v = (x8 + dx8) @ (wv8 + dwv8), dropping the dx*dwv
                    # cross term: 3 fp8 DoubleRow passes accumulate in PSUM;
                    # the x64 weight pre-scale is undone in the evacuation
                    for tb in range(NB):
                        ps = ps_m.tile([128, 512], f32, name="psmm", tag="psmm")
                        xq = x8[tb // 4][:, :, 128 * (tb % 4):
                                         128 * (tb % 4 + 1)]
                        xd = xd8[tb // 8][:, :, 128 * (tb % 8):
                                          128 * (tb % 8 + 1)]
                        passes = [(xq, wv_s), (xd, wv_s), (xq, wvd_s)]
                        n = 0
                        for xi, wi in passes:
                            for kk2 in range(0, NKK, 2):
                                n += 1
                                nc.tensor.matmul(
                                    ps[:, 0:256],
                                    xi[:, kk2:kk2 + 2, :],
                                    wi[:, kk2:kk2 + 2, :],
                                    start=(n == 1),
                                    stop=(n == 3 * NKK // 2),
                                    perf_mode=DR,
                                )
                        nc.vector.tensor_scalar_mul(
                            v4[:, tb, :, 0:64],
                            ps[:, 0:256].rearrange("p (s d) -> p s d", d=64),
                            1.0 / 64.0)

                # ---- attention, split into score (A) and context (B)
                # phases so the emission (= in-order engine queue) order can
                # overlap the big slots' exp with projection matmuls ----
                def attn_scores(s, IC, ps_s, pt_maps):
                    d = D_KEEP[s]
                    kr = slice(0, 68) if s % 2 == 0 else slice(0, 128)

                    if True:
                        ic_lo, ic_hi = 1024 * IC, 1024 * (IC + 1)
                        # Collect each j0's kept i-window, split into pieces
                        # of >= 256 cols (fp32r matmuls run 1 cycle/row only
                        # at ap_size >= 256; a handful of 128-wide windows
                        # are unavoidable), then pack the pieces exactly into
                        # 512-col PSUM banks, pairs of banks forming the
                        # shared [128,1024] tiles a single exp covers. Exact
                        # bank packing wastes no exp columns.
                        pieces = []
                        for j0 in range(NB):
                            lo = max(ic_lo, 128 * j0)
                            hi = min(ic_hi, 128 * (j0 + d), T)
                            rem = hi - lo
                            while rem > 0:
                                take = (rem if rem <= 512
                                        else 384 if rem == 640 else 512)
                                pieces.append((j0, lo, take))
                                lo += take
                                rem -= take
                        # first-fit-decreasing into 512-col banks
                        banks = []
                        for j0, lo, w in sorted(pieces, key=lambda r: -r[2]):
                            for b in banks:
                                if b[0] + w <= 512:
                                    b[1].append((j0, lo, w, b[0]))
                                    b[0] += w
                                    break
                            else:
                                banks.append([w, [(j0, lo, w, 0)]])
                        # full banks first so each [128,1024] tile's used
                        # region [0:fill] stays contiguous for the exp
                        banks.sort(key=lambda b: -b[0])
                        bins, fills = [], []
                        for bi in range(0, len(banks), 2):
                            pair = banks[bi:bi + 2]
                            regs = [(j0, lo, w, ofs) for j0, lo, w, ofs
                                    in pair[0][1]]
                            fill = pair[0][0]
                            if len(pair) > 1:
                                regs += [(j0, lo, w, 512 + ofs)
                                         for j0, lo, w, ofs in pair[1][1]]
                                fill = 512 + pair[1][0]
                            bins.append(regs)
                            fills.append(fill)
                        for bin_regions, fill in zip(bins, fills):
                            sps = ps_s.tile([128, 1024], f32, name="sps",
                                            tag="sps")
                            for j0, lo, w, ofs in bin_regions:
                                # matmul output must stay within one PSUM
                                # bank: split pieces at 512 boundaries
                                p0 = ofs
                                while p0 < ofs + w:
                                    p1 = min(ofs + w, (p0 // 512 + 1) * 512)
                                    nc.tensor.matmul(
                                        sps[:, p0:p1],
                                        k_att[s][kr, 128 * j0:128 * (j0 + 1)],
                                        q_att[s][kr, lo + (p0 - ofs):
                                                 lo + (p1 - ofs)],
                                        start=True, stop=True,
                                    )
                                    p0 = p1

                            pt = ptp.tile([128, 1024], bf16, name="pt",
                                          tag="pt")
                            nc.scalar.activation(pt[:, 0:fill], sps[:, 0:fill],
                                                 Exp, bias=zbias[:])
                            for j0, lo, w, ofs in bin_regions:
                                if lo == 128 * j0:
                                    # causal mask on the diagonal block:
                                    # min with {causal side: inf, else: 0}
                                    # zeroes the masked (overflowed) entries;
                                    # bf16 SBUF-only op gets the DVE 4x mode
                                    nc.vector.tensor_tensor(
                                        pt[:, ofs:ofs + 128],
                                        pt[:, ofs:ofs + 128], mcl_s[:],
                                        mybir.AluOpType.min)
                                pt_maps[s].setdefault((IC, j0), []).append(
                                    (pt, lo, w, ofs))

                def attn_ctx_multi(slots, IC, ps_c, pt_maps, fuse=None):
                    # context matmuls + softmax divides, interleaved across
                    # slots per row-block so the fused out-proj for block i0
                    # can start as soon as every slot has divided i0. All
                    # four slots write disjoint 65-col ranges of one PSUM
                    # tile so a single strided reciprocal covers the four
                    # denominators.
                    for i0 in range(8 * IC, 8 * (IC + 1)):
                        pcx = ps_c.tile([128, 4, 65], f32, name="pcx",
                                        tag="pcx")
                        for si, s in enumerate(slots):
                            d = D_KEEP[s]
                            j_lo = max(0, i0 - d + 1)
                            for j0 in range(j_lo, i0 + 1):
                                col = 128 * i0
                                for pt, lo, w, ofs in pt_maps[s][(IC, j0)]:
                                    if lo <= col < lo + w:
                                        break
                                else:
                                    raise AssertionError((s, IC, j0, i0))
                                off = ofs + col - lo
                                nc.tensor.matmul(
                                    pcx[:, si, :],
                                    pt[:, off:off + 128],
                                    v4[:, j0, s, :],
                                    start=(j0 == j_lo), stop=(j0 == i0),
                                )
                        rc = small.tile([128, 4], f32, name="rc", tag="rc")
                        nc.vector.reciprocal(rc[:], pcx[:, :, 64])
                        for si, s in enumerate(slots):
                            nc.vector.tensor_scalar_mul(
                                ctx_sb[:, i0, 64 * s:64 * (s + 1)],
                                pcx[:, si, 0:64], rc[:, si:si + 1])
                        if fuse is not None:
                            fuse(i0)

                # ---- emission ----
                with ExitStack() as attn_ctx:
                    ps_s = attn_ctx.enter_context(
                        tc.tile_pool(name="ps_s", bufs=2, space="PSUM"))
                    ps_c = attn_ctx.enter_context(
                        tc.tile_pool(name="ps_c", bufs=2, space="PSUM"))
                    pt_maps = [{} for _ in range(HPC)]
                    alibi_dmas(0)
                    alibi_dmas(1)
                    for tcq in range(NTC):
                        proj_qk_tc(wq_s, bq_s, q_att, 0, tcq)
                        proj_qk_tc(wk_s, bk_s, k_att, 0, tcq)
                    alibi_dmas(2)
                    alibi_dmas(3)
                    late_dmas()
                    attn_scores(0, 0, ps_s, pt_maps)
                    attn_scores(1, 0, ps_s, pt_maps)
                    for tcq in range(NTC):
                        proj_qk_tc(wq_s, bq_s, q_att, 1, tcq)
                        proj_qk_tc(wk_s, bk_s, k_att, 1, tcq)
                    attn_scores(0, 1, ps_s, pt_maps)
                    attn_scores(2, 0, ps_s, pt_maps)
                    attn_scores(3, 0, ps_s, pt_maps)
                    proj_v()

                    # ---- output projection, fused into the context phase.
                    # Out-proj block tb only needs ctx_sb[:, tb, :], whose
                    # last writer is the final slot's divide for i0 == tb, so
                    # the matmuls slot in right after it.
                    # Transposes borrow ps_c slots, matmuls reuse ps_m.
                    def op_transpose(tb):
                        ctts = []
                        for k in range(2):
                            pst = ps_m.tile([128, 128], bf16, name="pst",
                                            tag="psmm")
                            nc.tensor.transpose(
                                pst[:], ctx_sb[:, tb, 128 * k:128 * (k + 1)],
                                id_s[:])
                            ctt = small.tile([128, 128], bf16, name="ctt",
                                             tag="ctt")
                            nc.vector.tensor_copy(ctt[:], pst[:])
                            ctts.append(ctt)
                        return ctts

                    def op_mms(tb, ctts):
                        ob = obp.tile([128, 1024], bf16, name="ob",
                                      tag="ob")
                        for oc in range(2):
                            po = ps_m.tile([128, 512], f32, name="po",
                                           tag="psmm")
                            for k in range(2):
                                nc.tensor.matmul(
                                    po[:],
                                    ctts[k][:],
                                    wp_s[:, k, 512 * oc:512 * (oc + 1)],
                                    start=(k == 0), stop=(k == 1),
                                )
                            sl = slice(512 * oc, 512 * (oc + 1))
                            if (oc == 0) != (tb % 2 == 0):
                                nc.scalar.activation(ob[:, sl], po[:], Copy)
                            else:
                                nc.vector.tensor_copy(ob[:, sl], po[:])
                        dma_eng = nc.sync if tb % 2 == 0 else nc.scalar
                        dma_eng.dma_start(
                            out_d[128 * tb:128 * (tb + 1), :], ob[:])

                    # run the matmuls one block behind the transposes so the
                    # DVE copy of block tb hides under other PE work
                    pending = []

                    def outproj_tb(tb):
                        if pending:
                            op_mms(*pending.pop())
                        pending.append((tb, op_transpose(tb)))

                    if fuse_out:
                        attn_ctx_multi([0, 2, 3, 1], 0, ps_c, pt_maps,
                                       fuse=outproj_tb)
                        attn_scores(2, 1, ps_s, pt_maps)
                        attn_scores(3, 1, ps_s, pt_maps)
                        attn_scores(1, 1, ps_s, pt_maps)
                        attn_ctx_multi([0, 2, 3, 1], 1, ps_c, pt_maps,
                                       fuse=outproj_tb)
                        op_mms(*pending.pop())
                    else:
                        attn_ctx_multi([0, 2, 3, 1], 0, ps_c, pt_maps)
                        attn_scores(2, 1, ps_s, pt_maps)
                        attn_scores(3, 1, ps_s, pt_maps)
                        attn_scores(1, 1, ps_s, pt_maps)
                        attn_ctx_multi([0, 2, 3, 1], 1, ps_c, pt_maps)
                        for tb in range(NB):
                            outproj_tb(tb)
                        op_mms(*pending.pop())


            for _rep in range(nrep):
                emit_once()

    nc.compile()
    return nc


def _prep_core_inputs(core, x, Wq, bq, Wk, bk, Wv):
    b, c = core // HPC, core % HPC
    heads = [c + HPC * (HPC - 1 - s) for s in range(HPC)]
    sl = np.concatenate([np.arange(h * HD, (h + 1) * HD) for h in heads])
    slopes = 2.0 ** (-8.0 * (np.asarray(heads, np.float64) + 1) / NH)
    pos = np.arange(T, dtype=np.float32)

    FP8 = ml_dtypes.float8_e4m3
    xTf = np.ascontiguousarray(x[b].T).astype(np.float32)
    x8 = xTf.astype(FP8)
    xd8 = (xTf - x8.astype(np.float32)).astype(FP8)
    # x64 pre-scale keeps the fp8 weight mantissas in the normal range;
    # the kernel's PSUM evacuations multiply by 1/64
    wq = np.ascontiguousarray((64.0 * Wq[sl] / SCALE).T).astype(FP8)
    wk = np.ascontiguousarray((64.0 * Wk[sl] / SCALE).T).astype(FP8)
    wvf = np.ascontiguousarray(64.0 * Wv[sl].T).astype(np.float32)
    wv = wvf.astype(FP8)
    wvd = (wvf - wv.astype(np.float32)).astype(FP8)

    def bias_cols(vec):
        # [128, HPC]: even slot s -> rows 0:64, odd slot s -> rows 64:128
        cols = np.zeros((128, HPC), np.float32)
        per_slot = vec.reshape(HPC, HD)
        for s in range(HPC):
            r0 = 0 if s % 2 == 0 else 64
            cols[r0:r0 + 64, s] = per_slot[s]
        return cols

    bq_c = bias_cols((bq[sl] / SCALE).astype(np.float32))
    bk_c = bias_cols(bk[sl].astype(np.float32))
    def hi_lo(a):
        # split into bf16-exact hi + small lo so each part is (nearly)
        # exactly representable in the PE's fp32r decomposition
        hi = a.astype(BF16).astype(np.float32)
        lo = (a - hi).astype(np.float32)
        lo_hi = lo.astype(BF16).astype(np.float32)
        lo = lo_hi + ((lo - lo_hi).astype(BF16).astype(np.float32))
        return hi, lo

    ones = np.ones(T, np.float32)
    qrows, krows = [], []
    for s in range(HPC):
        bhi, blo = hi_lo((-slopes[s] * pos).astype(np.float32))
        ahi, alo = hi_lo((slopes[s] * pos).astype(np.float32))
        qrows.append(np.stack([ones, ones, bhi, blo]))
        krows.append(np.stack([ahi, alo, ones, ones]))
    qrows = np.stack(qrows).astype(np.float32)
    krows = np.stack(krows).astype(np.float32)
    # post-exp clamp tile for the diagonal block: min(exp, clamp) keeps the
    # causal side (clamp=inf) and zeroes the masked side (clamp=0), where the
    # exp has overflowed to inf
    jj = np.arange(128)
    maskadd = np.where(jj[:, None] <= jj[None, :], np.inf, 0.0).astype(BF16)
    ident = np.eye(128, dtype=BF16)
    return {
        "x8": x8, "xd8": xd8, "wq": wq, "wk": wk, "wv": wv, "wvd": wvd,
        "bq": bq_c, "bk": bk_c,
        "qrows": qrows, "krows": krows, "maskadd": maskadd, "ident": ident,
    }


def _prep_wp(core, Wp):
    c = core % HPC
    heads = [c + HPC * (HPC - 1 - s) for s in range(HPC)]
    sl = np.concatenate([np.arange(h * HD, (h + 1) * HD) for h in heads])
    return np.ascontiguousarray(Wp[:, sl].T).astype(BF16)


def _run(inputs, trace=False):
    from concourse.bass_utils import run_bass_kernel_spmd

    global _PROG
    if _PROG is None:
        _PROG = _build_program()

    x = np.asarray(inputs["x"], np.float32)
    Wq = np.asarray(inputs["Wq"], np.float32)
    bq = np.asarray(inputs["bq"], np.float32)
    Wk = np.asarray(inputs["Wk"], np.float32)
    bk = np.asarray(inputs["bk"], np.float32)
    Wv = np.asarray(inputs["Wv"], np.float32)
    bv = np.asarray(inputs["bv"], np.float32)
    Wp = np.asarray(inputs["Wp"], np.float32)
    bp = np.asarray(inputs["bp"], np.float32)
    assert int(inputs["num_heads"]) == NH

    in_maps = []
    for core in range(8):
        m = _prep_core_inputs(core, x, Wq, bq, Wk, bk, Wv)
        m["wp"] = _prep_wp(core, Wp)
        in_maps.append(m)

    res = run_bass_kernel_spmd(_PROG, in_maps, core_ids=list(range(8)),
                               trace=trace)
    out = np.zeros((B, T, D), np.float32)
    for core in range(8):
        out[core // HPC] += np.asarray(res.results[core]["out"], np.float32)
    out += (bp + bv @ Wp.T)[None, None, :]
    return out, res


def kernel(**inputs) -> np.ndarray:
    out, _ = _run(inputs, trace=False)
    return out

